# revision 2
# baseline (speedup 1.0000x reference)
"""Trainium2 Bass kernel v5: wave-interleaved Linformer attention
(v3 structure) with fp8 Q/K projections, bf16 output, and startup fixes.

B=4, T=4096, C=1024, H=16, HS=64, K=256.
Sharding: 8 cores = batch (4) x head-group (2 groups of 8 heads).

v3 over v2: the 8 heads per core are split into 2 waves of 4. Schedule:
  phase A: qkv+kpvp (wave 0), all t-blocks
  phase B: qkv+kpvp (wave 1) interleaved with attention (wave 0)
  phase C: attention (wave 1) + output projection (both waves)
Wave 0's attT tiles are held in SBUF across B/C. The interleave lets the
PE fill the softmax chain's dependency stalls with the other wave's
projection matmuls. PSUM is exactly 8 banks: K+V share one bank per sub,
and the output-projection PSUM reuses the Q-projection pool.
x is re-streamed from HBM for wave 1 (SBUF can't hold it resident).
kpT/vpT accumulate across 2-t-block windows in a single PSUM group
(halves the DVE drain adds); in phase B each t-block emits wave-0
attention before wave-1 projections; phase C borrows the idle phase-1
PSUM banks to double-buffer the softmax pipeline and trails the
projection one t-block behind the attention that feeds it.
NOTE: tile_position packed matmuls crash this runtime -- padded pair
tiles only.
"""
import sys
for p in ('/opt/trn_rl_repo', '/root/.axon_site/_ro/trn_rl_repo'):
    if p not in sys.path:
        sys.path.insert(0, p)

from contextlib import ExitStack

import numpy as np

import concourse.bacc as bacc
import concourse.mybir as mybir
from concourse import tile
from concourse.bass_utils import run_bass_kernel_spmd

f32 = mybir.dt.float32
f32r = mybir.dt.float32r
bf16 = mybir.dt.bfloat16
f8 = mybir.dt.float8e4
AF = mybir.ActivationFunctionType
ALU = mybir.AluOpType
DR = mybir.MatmulPerfMode.DoubleRow

B, T, C = 4, 4096, 1024
H, HS = 16, 64
K = 256
HL = 8            # heads per core
HW_ = 4           # heads per wave
TB = 512          # t-block
NTB = T // TB     # 8
NC_ = C // 128    # 8 c-chunks
SCALE = 1.0 / np.sqrt(np.float32(K))  # 1/16


def _build_program(phases=3, repeat=1, timing=False):
    nc = bacc.Bacc("TRN2", target_bir_lowering=False, debug=False, num_devices=8)

    if timing:
        DIN = nc.declare_dram_parameter("DIN", [128, 128], f32, isOutput=False)
        DOUT = nc.declare_dram_parameter("DOUT", [128, 128], f32, isOutput=True)
        decl = lambda name, shape, dt_, out=False: nc.dram_tensor(name, shape, dt_)
    else:
        decl = lambda name, shape, dt_, out=False: nc.declare_dram_parameter(
            name, shape, dt_, isOutput=out)
    XT = decl("XT", [C, T], bf16)
    XT8 = decl("XT8", [C, T], f8)             # fp8 copy of x^T for Q/K paths
    WQ8 = decl("WQ8", [128, 4 * 1024], f8)    # host-packed [128, cp, two, 512]
    WK8 = decl("WK8", [128, 4 * 1024], f8)    # host-packed [128, cp, two, 512]
    WV = decl("WV", [128, NC_ * 512], bf16)
    ED = decl("ED", [HL, T, K], bf16)
    WPT = decl("WPT", [128, 4 * C], bf16)     # host-packed: ci-chunk m at cols m*C
    MSK = decl("MSK", [2, 128, K], f32)
    IDN = decl("IDN", [128, 128], f32)
    O = decl("O", [T, C], bf16, out=True)

    with tile.TileContext(nc) as tc, ExitStack() as top:
        # ---- persistent SBUF pools ----
        misc = top.enter_context(tc.tile_pool(name="misc", bufs=1))
        wp_p = top.enter_context(tc.tile_pool(name="wp", bufs=1))
        w_p = top.enter_context(tc.tile_pool(name="w", bufs=1))
        kvacc_p = top.enter_context(tc.tile_pool(name="kvacc", bufs=1))
        vp_p = top.enter_context(tc.tile_pool(name="vp", bufs=1))
        qres_p = top.enter_context(tc.tile_pool(name="qres", bufs=1))
        xt_p = top.enter_context(tc.tile_pool(name="xt", bufs=12))
        x8_p = top.enter_context(tc.tile_pool(name="x8", bufs=2))
        e_p = top.enter_context(tc.tile_pool(name="e", bufs=8))
        kv_p = top.enter_context(tc.tile_pool(name="kv", bufs=2))
        ew_p = top.enter_context(tc.tile_pool(name="ew", bufs=5))
        zz_p = top.enter_context(tc.tile_pool(name="zz", bufs=4))
        wn_p = top.enter_context(tc.tile_pool(name="wn", bufs=7))
        wt_p = top.enter_context(tc.tile_pool(name="wt", bufs=4))
        at_p = top.enter_context(tc.tile_pool(name="at", bufs=2))
        aw_p = top.enter_context(tc.tile_pool(name="aw", bufs=1))
        atmp_p = top.enter_context(tc.tile_pool(name="atmp", bufs=3))
        out_p = top.enter_context(tc.tile_pool(name="outp", bufs=3))
        # ---- PSUM pools: exactly 8 banks ----
        psq_p = top.enter_context(tc.tile_pool(name="psq", bufs=2, space="PSUM"))
        pskv2_p = top.enter_context(tc.tile_pool(name="pskv2", bufs=2, space="PSUM"))
        pse_p = top.enter_context(tc.tile_pool(name="pse", bufs=1, space="PSUM"))
        pss_p = top.enter_context(tc.tile_pool(name="pss", bufs=1, space="PSUM"))
        pswt_p = top.enter_context(tc.tile_pool(name="pswt", bufs=1, space="PSUM"))
        pso_p = top.enter_context(tc.tile_pool(name="pso", bufs=1, space="PSUM"))

        identf = misc.tile([128, 128], f32, tag="identf", name="identf")
        nc.sync.dma_start(identf[:], IDN[:])
        ident = misc.tile([128, 128], f32r, tag="ident", name="ident")
        nc.vector.tensor_copy(ident[:].bitcast(f32r), identf[:])
        identb = misc.tile([128, 128], bf16, tag="identb", name="identb")
        nc.scalar.copy(identb[:], identf[:])
        masksb = []
        for i in range(2):
            mt = misc.tile([128, K], f32, tag=f"msk{i}", name=f"msk{i}")
            nc.sync.dma_start(mt[:], MSK[i])
            masksb.append(mt)

        wq8 = w_p.tile([128, 4, 2, 512], f8, tag="wq8", name="wq8")
        wk8 = w_p.tile([128, 4, 2, 512], f8, tag="wk8", name="wk8")
        wvt = w_p.tile([128, NC_ * 512], bf16, tag="wvt", name="wvt")
        wpt = wp_p.tile([128, 4 * C], bf16, tag="wpt", name="wpt")

        kvacc = [kvacc_p.tile([128, K], f32, tag=f"kvacc{h}", name=f"kvacc{h}")
                 for h in range(HL)]
        vppair = [[vp_p.tile([128, 128], bf16, tag=f"vpp{pr}_{j}",
                             name=f"vpp{pr}_{j}") for j in range(2)]
                  for pr in range(4)]
        kpb = [vp_p.tile([128, 2 * K], bf16, tag=f"kpb{pr}", name=f"kpb{pr}")
               for pr in range(4)]
        qres = [qres_p.tile([128, T], bf16, tag=f"qres{m}", name=f"qres{m}")
                for m in range(4)]
        # wave-0 attT storage, alive from phase B into phase C
        attw0 = [[aw_p.tile([128, TB], bf16, tag=f"aw{tb}_{pl}",
                            name=f"aw{tb}_{pl}") for pl in range(2)]
                 for tb in range(NTB)]

        xtt = [None] * NC_
        xt8 = [None]
        ett = [None] * HW_

        def phase1_tb(w, tb):
            """qkv + kpvp for wave w (heads 4w..4w+4), t-block tb."""
            t0 = tb * TB
            tbo = tb % 2
            if tbo == 0:
                # x8 first (feeds the immediate Q/K matmuls), then the bf16
                # chunks (V), then E (kpvp, consumed last)
                x8 = x8_p.tile([128, 4, 2, 2 * TB], f8, tag="x8", name="x8")
                nc.sync.dma_start(x8[:], XT8[:, t0:t0 + 2 * TB].rearrange(
                    "(cp two p) t -> p cp two t", p=128, two=2))
                xt8[0] = x8
                for c in range(NC_):
                    x_t = xt_p.tile([128, 2 * TB], bf16, tag="xt", name="xt")
                    nc.sync.dma_start(x_t[:], XT[c * 128:(c + 1) * 128,
                                                 t0:t0 + 2 * TB])
                    xtt[c] = x_t
                for hl in range(HW_):
                    e_t = e_p.tile([128, 8, K], bf16, tag="et", name="et")
                    src = ED[4 * w + hl, t0:t0 + 2 * TB, :].rearrange(
                        "(s p) r -> p s r", p=128)
                    nc.sync.dma_start(e_t[:], src)
                    ett[hl] = e_t

            # Q projection for the wave's two head-pairs (fp8 DoubleRow)
            x8 = xt8[0]
            for m in (2 * w, 2 * w + 1):
                psq = psq_p.tile([128, 512], f32, tag="psq", name="psq")
                for cp in range(4):
                    nc.tensor.matmul(psq[:],
                                     wq8[:, cp, :, m * 128:(m + 1) * 128],
                                     x8[:, cp, :, tbo * TB:(tbo + 1) * TB],
                                     start=(cp == 0), stop=(cp == 3),
                                     perf_mode=DR)
                if m % 2 == 0:
                    nc.scalar.copy(qres[m][:, t0:t0 + TB], psq[:])
                else:
                    nc.vector.tensor_copy(qres[m][:, t0:t0 + TB], psq[:])

            # K,V projections: K in cols 0:256, V in 256:512 of one bank
            kvsb = []
            for sub in range(4):
                ts0 = tbo * TB + sub * 128
                pskv2 = pskv2_p.tile([128, 512], f32, tag="pskv2", name="pskv2")
                for cp in range(4):
                    nc.tensor.matmul(pskv2[:, 0:256],
                                     x8[:, cp, :, ts0:ts0 + 128],
                                     wk8[:, cp, :, w * 256:w * 256 + 256],
                                     start=(cp == 0), stop=(cp == 3),
                                     perf_mode=DR)
                for c in range(NC_):
                    nc.tensor.matmul(pskv2[:, 256:512],
                                     xtt[c][:, tbo * TB + sub * 128:tbo * TB + (sub + 1) * 128],
                                     wvt[:, c * 512 + w * 256:c * 512 + w * 256 + 256],
                                     start=(c == 0), stop=(c == NC_ - 1))
                kvt = kv_p.tile([128, 512], bf16, tag=f"kv{sub}", name=f"kv{sub}")
                kv4 = kvt[:].rearrange("p (hp x s) -> p hp x s", hp=2, x=4, s=HS)
                psk4 = pskv2[:, 0:256].rearrange("p (hp e s) -> p hp e s",
                                                 hp=2, e=2, s=HS)
                psv4 = pskv2[:, 256:512].rearrange("p (hp e s) -> p hp e s",
                                                   hp=2, e=2, s=HS)
                nc.scalar.copy(kv4[:, :, 0, :], psk4[:, :, 0, :])
                nc.vector.tensor_copy(kv4[:, :, 1, :], psv4[:, :, 0, :])
                nc.scalar.copy(kv4[:, :, 2, :], psv4[:, :, 1, :])
                nc.vector.tensor_copy(kv4[:, :, 3, :], psk4[:, :, 1, :])
                kvsb.append(kvt)

            return kvsb

        def kpvp_pair(w, tb_odd, kvsb0, kvsb1):
            """kpT/vpT for a 2-t-block window in one PSUM group per head --
            halves the DVE drain adds. In phase A (wave 0) the idle pswt bank
            double-buffers the accumulators."""
            for hl in range(HW_):
                h = 4 * w + hl
                pp = pswt_p if (w == 0 and hl % 2 == 1) else pse_p
                pse = pp.tile([128, K], f32,
                              tag="pswt" if pp is pswt_p else "pse", name="pse")
                for s8 in range(8):
                    kvt = (kvsb0 if s8 < 4 else kvsb1)[s8 % 4]
                    nc.tensor.matmul(pse[:], kvt[:, hl * 128:(hl + 1) * 128],
                                     ett[hl][:, s8, :],
                                     start=(s8 == 0), stop=(s8 == 7))
                if tb_odd == 1:
                    nc.vector.tensor_copy(kvacc[h][:].bitcast(f32r), pse[:])
                else:
                    nc.vector.tensor_tensor(kvacc[h][:].bitcast(f32r), kvacc[h][:],
                                            pse[:], op=ALU.add)

        def phase15(w):
            """kpb (padded bf16) + vppair via PE transpose, wave w."""
            for pl in range(2):
                pr = 2 * w + pl
                h0, h1 = 2 * pr, 2 * pr + 1
                nc.gpsimd.memset(kpb[pr][HS:128, 0:K], 0.0)
                nc.gpsimd.memset(kpb[pr][0:HS, K:2 * K], 0.0)
                nc.scalar.copy(kpb[pr][0:HS, 0:K], kvacc[h0][0:HS, :])
                nc.scalar.copy(kpb[pr][HS:128, K:2 * K], kvacc[h1][HS:128, :])
                for h01 in range(2):
                    h = 2 * pr + h01
                    lo = (h01 == 0)
                    for j in range(2):
                        pvp = pse_p if j == 0 else pswt_p
                        psvp = pvp.tile([128, K], f32,
                                        tag="pse" if pvp is pse_p else "pswt",
                                        name="psvp")
                        nc.tensor.transpose(psvp[:, 0:128].bitcast(f32r),
                                            kvacc[h][:, j * 128:(j + 1) * 128].bitcast(f32r),
                                            ident[:])
                        vcols = psvp[:, 64:128] if lo else psvp[:, 0:64]
                        if h01 == 0:
                            nc.scalar.copy(vppair[pr][j][:, 0:HS], vcols)
                        else:
                            nc.vector.tensor_copy(vppair[pr][j][:, HS:128], vcols)

        def phase2_tb(w, tb, attTs, borrow=False):
            """attention (S/softmax/A) for wave w, t-block tb.
            attTs: list of 2 destination [128, TB] bf16 tiles (one per pair).
            borrow: double-buffer pss/pswt out of the idle phase-1 pools."""
            for pl in range(2):
                pr = 2 * w + pl
                att_dst = attTs[pl]
                # wt4: [128, sub(4), h01*2+j(4), 128] bf16, filled per sub
                wt4 = wt_p.tile([128, 4, 4, 128], bf16, tag="wt4", name="wt4")
                for sub in range(4):
                    tt = tb * 4 + sub
                    pssp = pskv2_p if (borrow and sub % 2 == 1) else pss_p
                    pss = pssp.tile([128, 2 * K], f32,
                                    tag="pskv2" if pssp is pskv2_p else "pss",
                                    name="pss")
                    nc.tensor.matmul(pss[:],
                                     qres[pr][:, tt * 128:(tt + 1) * 128],
                                     kpb[pr][:], start=True, stop=True)
                    if tt < 2:
                        for h01 in range(2):
                            nc.vector.tensor_tensor(
                                pss[:, h01 * K:(h01 + 1) * K],
                                pss[:, h01 * K:(h01 + 1) * K],
                                masksb[tt][:], op=ALU.add)
                    expw = ew_p.tile([128, 2 * K], bf16, tag="expw", name="expw")
                    z = zz_p.tile([128, 2], f32, tag="z", name="z")
                    rec = zz_p.tile([128, 2], f32, tag="rec", name="rec")
                    if sub % 2 == 0:
                        for h01 in range(2):
                            nc.scalar.activation(
                                expw[:, h01 * K:(h01 + 1) * K],
                                pss[:, h01 * K:(h01 + 1) * K],
                                AF.Exp, scale=float(SCALE),
                                accum_out=z[:, h01:h01 + 1])
                    else:
                        nc.scalar.activation(expw[:], pss[:], AF.Exp,
                                             scale=float(SCALE))
                        for h01 in range(2):
                            nc.vector.tensor_reduce(
                                z[:, h01:h01 + 1],
                                expw[:, h01 * K:(h01 + 1) * K],
                                axis=mybir.AxisListType.X, op=ALU.add)
                    nc.vector.reciprocal(rec[:], z[:])
                    pwp = pse_p if (borrow and sub % 2 == 0) else pswt_p
                    pswt = pwp.tile([128, 256], f32,
                                    tag="pse" if pwp is pse_p else "pswt",
                                    name="pswt")
                    psb = pswt[:].bitcast(bf16)
                    for h01 in range(2):
                        wn = wn_p.tile([128, K], bf16, tag="wn", name="wn")
                        nc.vector.tensor_scalar_mul(
                            wn[:], expw[:, h01 * K:(h01 + 1) * K],
                            rec[:, h01:h01 + 1])
                        for j in range(2):
                            nc.tensor.transpose(
                                psb[:, (h01 * 2 + j) * 128:(h01 * 2 + j + 1) * 128],
                                wn[:, j * 128:(j + 1) * 128],
                                identb[:])
                    nc.vector.tensor_copy(wt4[:, sub, :, :], psb.rearrange(
                        "p (hj i) -> p hj i", hj=4))
                for h01 in range(2):
                    pso = pso_p.tile([HS, TB], f32, tag="pso", name="pso")
                    for j in range(2):
                        nc.tensor.matmul(
                            pso[:],
                            vppair[pr][j][:, h01 * HS:(h01 + 1) * HS],
                            wt4[:, :, h01 * 2 + j, :],
                            start=(j == 0), stop=(j == 1))
                    if h01 == 0:
                        nc.vector.tensor_copy(att_dst[0:HS, :], pso[:])
                    else:
                        atmp = atmp_p.tile([HS, TB], bf16, tag="atmp", name="atmp")
                        nc.scalar.copy(atmp[:], pso[:])
                        nc.sync.dma_start(att_dst[HS:128, :], atmp[:])

        def proj_tb(tb, attT4):
            t0 = tb * TB
            last = (tb == NTB - 1)
            for sub in range(4):
                outsb = out_p.tile([128, C], bf16, tag="outsb", name="outsb")
                row = t0 + sub * 128
                for n in range(2):
                    psp = psq_p.tile([128, 512], f32, tag="psq", name="psp")
                    for ci in range(4):
                        nc.tensor.matmul(psp[:],
                                         attT4[ci][:, sub * 128:(sub + 1) * 128],
                                         wpt[:, ci * C + n * 512:ci * C + (n + 1) * 512],
                                         start=(ci == 0), stop=(ci == 3))
                    if n == 0:
                        nc.scalar.copy(outsb[:, 0:512], psp[:])
                    else:
                        nc.vector.tensor_copy(outsb[:, 512:1024], psp[:])
                    if last:
                        # drain the final rows in halves on alternating
                        # queues so the tail DMA starts as soon as each
                        # half is staged
                        eng = nc.sync if n == 0 else nc.scalar
                        eng.dma_start(O[row:row + 128, n * 512:(n + 1) * 512],
                                      outsb[:, n * 512:(n + 1) * 512])
                if not last:
                    nc.sync.dma_start(O[row:row + 128, :], outsb[:])

        for _rep in range(max(1, repeat)):
            # halves so the first Q matmuls start ~1us earlier; wpt is
            # first read in phase C so it loads during phase B instead of
            # stealing phase-A startup DMA bandwidth
            wq8f = wq8[:].rearrange("p cp two m -> p (cp two m)")
            nc.scalar.dma_start(wq8f[:, 0:2048], WQ8[:, 0:2048])
            nc.scalar.dma_start(wq8f[:, 2048:], WQ8[:, 2048:])
            nc.scalar.dma_start(wk8[:].rearrange("p cp two m -> p (cp two m)"),
                                WK8[:])
            nc.scalar.dma_start(wvt[:], WV[:])
            if phases & 1:
                # phase A: wave-0 projections
                kvprev = None
                for tb in range(NTB):
                    kvsb = phase1_tb(0, tb)
                    if tb % 2 == 1:
                        kpvp_pair(0, tb, kvprev, kvsb)
                    kvprev = kvsb
                phase15(0)
            if not (phases & 1):
                nc.scalar.dma_start(wpt[:], WPT[:])
            if phases == 3:
                # phase B: wave-1 projections interleaved with wave-0 attention
                nc.scalar.dma_start(wpt[:], WPT[:])
                kvprev = None
                for tb in range(NTB):
                    phase2_tb(0, tb, attw0[tb])
                    kvsb = phase1_tb(1, tb)
                    if tb % 2 == 1:
                        kpvp_pair(1, tb, kvprev, kvsb)
                    kvprev = kvsb
                phase15(1)
            if phases & 2:
                # phase C: wave-1 attention + full projection (proj trails by
                # one t-block so it fills the next block's softmax stalls)
                prev = None
                for tb in range(NTB):
                    attT1 = [at_p.tile([128, TB], bf16, tag=f"attT{p}",
                                       name=f"attT{p}") for p in range(2)]
                    phase2_tb(1, tb, attT1, borrow=(phases == 3))
                    if prev is not None:
                        proj_tb(tb - 1, [attw0[tb - 1][0], attw0[tb - 1][1],
                                         prev[0], prev[1]])
                    prev = attT1
                proj_tb(NTB - 1, [attw0[NTB - 1][0], attw0[NTB - 1][1],
                                  prev[0], prev[1]])

    nc.finalize()
    return nc


_NC_CACHE = {}


def _get_program(phases=3):
    if phases not in _NC_CACHE:
        _NC_CACHE[phases] = _build_program(phases)
    return _NC_CACHE[phases]


def _pack_w(w_core):
    """[C, 512] -> [128, 8*512] with chunk c at cols c*512."""
    return np.ascontiguousarray(
        w_core.reshape(NC_, 128, 512).transpose(1, 0, 2).reshape(128, NC_ * 512))


def _pack_w8(w_core):
    """[C, 512] fp8 -> [128, cp(4), two(2), 512] flat [128, 4096]."""
    return np.ascontiguousarray(
        w_core.reshape(4, 2, 128, 512).transpose(2, 0, 1, 3).reshape(128, 4096))


def _make_in_maps(x, WQ, WK, WV, E, Wp):
    import ml_dtypes
    f8np = ml_dtypes.float8_e4m3
    xr = np.transpose(np.asarray(x), (0, 2, 1)).astype(ml_dtypes.bfloat16)  # [B, C, T]
    xr8 = np.transpose(np.asarray(x), (0, 2, 1)).astype(f8np)
    wq_full = np.transpose(np.asarray(WQ), (1, 0, 2)).astype(f8np)
    wk_full = np.transpose(np.asarray(WK), (1, 0, 2)).astype(f8np)
    wv_full = np.transpose(np.asarray(WV), (1, 0, 2)).astype(ml_dtypes.bfloat16)
    er = np.asarray(E).astype(ml_dtypes.bfloat16)                 # [H, B, T, K]
    wpt_full = np.ascontiguousarray(np.asarray(Wp).T).astype(ml_dtypes.bfloat16)

    msk = np.zeros((2, 128, K), np.float32)
    for i in range(2):
        t_idx = i * 128 + np.arange(128)[:, None]
        msk[i] = np.where(np.arange(K)[None, :] <= t_idx, 0.0, -1e30)
    idn = np.eye(128, dtype=np.float32)

    in_maps = []
    for core in range(8):
        b, g = core // 2, core % 2
        hs = slice(g * HL, (g + 1) * HL)
        wpt_core = wpt_full[g * 512:(g + 1) * 512, :]              # [512, 1024]
        wpt_packed = np.ascontiguousarray(
            wpt_core.reshape(4, 128, C).transpose(1, 0, 2).reshape(128, 4 * C))
        in_maps.append({
            "XT": np.ascontiguousarray(xr[b]),
            "XT8": np.ascontiguousarray(xr8[b]),
            "WQ8": _pack_w8(np.ascontiguousarray(wq_full[:, hs, :]).reshape(C, HL * HS)),
            "WK8": _pack_w8(np.ascontiguousarray(wk_full[:, hs, :]).reshape(C, HL * HS)),
            "WV": _pack_w(np.ascontiguousarray(wv_full[:, hs, :]).reshape(C, HL * HS)),
            "ED": np.ascontiguousarray(er[hs, b]),
            "WPT": wpt_packed,
            "MSK": msk,
            "IDN": idn,
        })
    return in_maps


def _run(x, WQ, WK, WV, E, Wp, bp, trace=False):
    nc = _get_program()
    in_maps = _make_in_maps(x, WQ, WK, WV, E, Wp)
    kw = {}
    if trace:
        kw = dict(trace=True, trace_cores=[0])
    res = run_bass_kernel_spmd(nc, in_maps, list(range(8)), **kw)
    out = np.zeros((B, T, C), np.float32)
    for b in range(B):
        out[b] = (res.results[2 * b]["O"].astype(np.float32)
                  + res.results[2 * b + 1]["O"].astype(np.float32))
    out += np.asarray(bp, np.float32)[None, None, :]
    return out, res


def kernel(x, WQ, WK, WV, E, Wp, bp):
    out, _ = _run(x, WQ, WK, WV, E, Wp, bp, trace=False)
    return out


def kernel_traced(x, WQ, WK, WV, E, Wp, bp):
    out, res = _run(x, WQ, WK, WV, E, Wp, bp, trace=True)
    return out, res



# revision 3
# speedup vs baseline: 1.1928x; 1.1928x over previous
"""Trainium2 Bass kernel v5: wave-interleaved Linformer attention
(v3 structure) with fp8 Q/K projections, bf16 output, and startup fixes.

B=4, T=4096, C=1024, H=16, HS=64, K=256.
Sharding: 8 cores = batch (4) x head-group (2 groups of 8 heads).

v3 over v2: the 8 heads per core are split into 2 waves of 4. Schedule:
  phase A: qkv+kpvp (wave 0), all t-blocks
  phase B: qkv+kpvp (wave 1) interleaved with attention (wave 0)
  phase C: attention (wave 1) + output projection (both waves)
Wave 0's attT tiles are held in SBUF across B/C. The interleave lets the
PE fill the softmax chain's dependency stalls with the other wave's
projection matmuls. PSUM is exactly 8 banks: K+V share one bank per sub,
and the output-projection PSUM reuses the Q-projection pool.
x is re-streamed from HBM for wave 1 (SBUF can't hold it resident).
kpT/vpT accumulate across 2-t-block windows in a single PSUM group
(halves the DVE drain adds); in phase B each t-block emits wave-0
attention before wave-1 projections; phase C borrows the idle phase-1
PSUM banks to double-buffer the softmax pipeline and trails the
projection one t-block behind the attention that feeds it.
NOTE: tile_position packed matmuls crash this runtime -- padded pair
tiles only.
"""
import sys
for p in ('/opt/trn_rl_repo', '/root/.axon_site/_ro/trn_rl_repo'):
    if p not in sys.path:
        sys.path.insert(0, p)

from contextlib import ExitStack

import numpy as np

import concourse.bacc as bacc
import concourse.mybir as mybir
from concourse import tile
from concourse.bass_utils import run_bass_kernel_spmd

f32 = mybir.dt.float32
f32r = mybir.dt.float32r
bf16 = mybir.dt.bfloat16
f8 = mybir.dt.float8e4
AF = mybir.ActivationFunctionType
ALU = mybir.AluOpType
DR = mybir.MatmulPerfMode.DoubleRow

B, T, C = 4, 4096, 1024
H, HS = 16, 64
K = 256
HL = 8            # heads per core
HW_ = 4           # heads per wave
TB = 512          # t-block
NTB = T // TB     # 8
NC_ = C // 128    # 8 c-chunks
SCALE = 1.0 / np.sqrt(np.float32(K))  # 1/16


def _build_program(phases=3, repeat=1, timing=False):
    nc = bacc.Bacc("TRN2", target_bir_lowering=False, debug=False, num_devices=8)

    if timing:
        DIN = nc.declare_dram_parameter("DIN", [128, 128], f32, isOutput=False)
        DOUT = nc.declare_dram_parameter("DOUT", [128, 128], f32, isOutput=True)
        decl = lambda name, shape, dt_, out=False: nc.dram_tensor(name, shape, dt_)
    else:
        decl = lambda name, shape, dt_, out=False: nc.declare_dram_parameter(
            name, shape, dt_, isOutput=out)
    # window-major host-repacked inputs: every DMA is 128 contiguous
    # >=4KB partition rows (large descriptors) instead of thousands of
    # 0.5-2KB strided ones
    XTW = decl("XTW", [4, 128, NC_ * 1024], bf16)   # [win, p, c*t]
    X8W = decl("X8W", [4, 128, 8 * 1024], f8)       # [win, p, cp*two*t]
    WQ8 = decl("WQ8", [128, 4 * 1024], f8)    # host-packed [128, cp, two, 512]
    WK8 = decl("WK8", [128, 4 * 1024], f8)    # host-packed [128, cp, two, 512]
    WV = decl("WV", [128, NC_ * 512], bf16)
    EDW = decl("EDW", [HL, 4, 128, 8 * K], bf16)  # [h, win, p, s*r]
    WPT = decl("WPT", [128, 4 * C], bf16)     # host-packed: ci-chunk m at cols m*C
    MSK = decl("MSK", [2, 128, K], f32)
    IDN = decl("IDN", [128, 128], f32)
    O = decl("O", [T, C], bf16, out=True)

    with tile.TileContext(nc) as tc, ExitStack() as top:
        # ---- persistent SBUF pools ----
        misc = top.enter_context(tc.tile_pool(name="misc", bufs=1))
        wp_p = top.enter_context(tc.tile_pool(name="wp", bufs=1))
        w_p = top.enter_context(tc.tile_pool(name="w", bufs=1))
        kvacc_p = top.enter_context(tc.tile_pool(name="kvacc", bufs=1))
        vp_p = top.enter_context(tc.tile_pool(name="vp", bufs=1))
        qres_p = top.enter_context(tc.tile_pool(name="qres", bufs=1))
        xt_p = top.enter_context(tc.tile_pool(name="xt", bufs=2))
        x8_p = top.enter_context(tc.tile_pool(name="x8", bufs=2))
        e_p = top.enter_context(tc.tile_pool(name="e", bufs=6))
        kv_p = top.enter_context(tc.tile_pool(name="kv", bufs=2))
        ew_p = top.enter_context(tc.tile_pool(name="ew", bufs=4))
        zz_p = top.enter_context(tc.tile_pool(name="zz", bufs=4))
        wn_p = top.enter_context(tc.tile_pool(name="wn", bufs=7))
        wt_p = top.enter_context(tc.tile_pool(name="wt", bufs=4))
        at_p = top.enter_context(tc.tile_pool(name="at", bufs=2))
        aw_p = top.enter_context(tc.tile_pool(name="aw", bufs=1))
        atmp_p = top.enter_context(tc.tile_pool(name="atmp", bufs=3))
        out_p = top.enter_context(tc.tile_pool(name="outp", bufs=3))
        # ---- PSUM pools: exactly 8 banks ----
        psq_p = top.enter_context(tc.tile_pool(name="psq", bufs=2, space="PSUM"))
        pskv2_p = top.enter_context(tc.tile_pool(name="pskv2", bufs=2, space="PSUM"))
        pse_p = top.enter_context(tc.tile_pool(name="pse", bufs=1, space="PSUM"))
        pss_p = top.enter_context(tc.tile_pool(name="pss", bufs=1, space="PSUM"))
        pswt_p = top.enter_context(tc.tile_pool(name="pswt", bufs=1, space="PSUM"))
        pso_p = top.enter_context(tc.tile_pool(name="pso", bufs=1, space="PSUM"))

        identf = misc.tile([128, 128], f32, tag="identf", name="identf")
        nc.sync.dma_start(identf[:], IDN[:])
        ident = misc.tile([128, 128], f32r, tag="ident", name="ident")
        nc.vector.tensor_copy(ident[:].bitcast(f32r), identf[:])
        identb = misc.tile([128, 128], bf16, tag="identb", name="identb")
        nc.scalar.copy(identb[:], identf[:])
        masksb = []
        for i in range(2):
            mt = misc.tile([128, K], f32, tag=f"msk{i}", name=f"msk{i}")
            nc.sync.dma_start(mt[:], MSK[i])
            masksb.append(mt)

        wq8 = w_p.tile([128, 4, 2, 512], f8, tag="wq8", name="wq8")
        wk8 = w_p.tile([128, 4, 2, 512], f8, tag="wk8", name="wk8")
        wvt = w_p.tile([128, NC_ * 512], bf16, tag="wvt", name="wvt")
        wpt = wp_p.tile([128, 4 * C], bf16, tag="wpt", name="wpt")

        kvacc = [kvacc_p.tile([128, K], f32, tag=f"kvacc{h}", name=f"kvacc{h}")
                 for h in range(HL)]
        vppair = [[vp_p.tile([128, 128], bf16, tag=f"vpp{pr}_{j}",
                             name=f"vpp{pr}_{j}") for j in range(2)]
                  for pr in range(4)]
        kpb = [vp_p.tile([128, 2 * K], bf16, tag=f"kpb{pr}", name=f"kpb{pr}")
               for pr in range(4)]
        qres = [qres_p.tile([128, T], bf16, tag=f"qres{m}", name=f"qres{m}")
                for m in range(4)]
        # wave-0 attT storage, alive from phase B into phase C
        attw0 = [[aw_p.tile([128, TB], bf16, tag=f"aw{tb}_{pl}",
                            name=f"aw{tb}_{pl}") for pl in range(2)]
                 for tb in range(NTB)]

        xtt = [None] * NC_
        xt8 = [None]
        ett = [None] * HW_

        def phase1_tb(w, tb):
            """qkv + kpvp for wave w (heads 4w..4w+4), t-block tb."""
            t0 = tb * TB
            tbo = tb % 2
            if tbo == 0:
                win = tb // 2
                # x8 first (feeds the immediate Q/K matmuls), then the bf16
                # window (V), then E (kpvp, consumed last)
                x8 = x8_p.tile([128, 4, 2, 2 * TB], f8, tag="x8", name="x8")
                nc.sync.dma_start(
                    x8[:].rearrange("p cp two t -> p (cp two t)"), X8W[win])
                xt8[0] = x8
                xw = xt_p.tile([128, NC_, 2 * TB], bf16, tag="xt", name="xw")
                nc.sync.dma_start(
                    xw[:].rearrange("p c t -> p (c t)"), XTW[win])
                for c in range(NC_):
                    xtt[c] = xw[:, c, :]
                for hl in range(HW_):
                    e_t = e_p.tile([128, 8, K], bf16, tag="et", name="et")
                    nc.sync.dma_start(
                        e_t[:].rearrange("p s r -> p (s r)"),
                        EDW[4 * w + hl, win])
                    ett[hl] = e_t

            # Q projection for the wave's two head-pairs (fp8 DoubleRow)
            x8 = xt8[0]
            for m in (2 * w, 2 * w + 1):
                psq = psq_p.tile([128, 512], f32, tag="psq", name="psq")
                for cp in range(4):
                    nc.tensor.matmul(psq[:],
                                     wq8[:, cp, :, m * 128:(m + 1) * 128],
                                     x8[:, cp, :, tbo * TB:(tbo + 1) * TB],
                                     start=(cp == 0), stop=(cp == 3),
                                     perf_mode=DR)
                if m % 2 == 0:
                    nc.scalar.copy(qres[m][:, t0:t0 + TB], psq[:])
                else:
                    nc.vector.tensor_copy(qres[m][:, t0:t0 + TB], psq[:])

            # K,V projections: K in cols 0:256, V in 256:512 of one bank
            kvsb = []
            for sub in range(4):
                ts0 = tbo * TB + sub * 128
                pskv2 = pskv2_p.tile([128, 512], f32, tag="pskv2", name="pskv2")
                for cp in range(4):
                    nc.tensor.matmul(pskv2[:, 0:256],
                                     x8[:, cp, :, ts0:ts0 + 128],
                                     wk8[:, cp, :, w * 256:w * 256 + 256],
                                     start=(cp == 0), stop=(cp == 3),
                                     perf_mode=DR)
                for c in range(NC_):
                    nc.tensor.matmul(pskv2[:, 256:512],
                                     xtt[c][:, tbo * TB + sub * 128:tbo * TB + (sub + 1) * 128],
                                     wvt[:, c * 512 + w * 256:c * 512 + w * 256 + 256],
                                     start=(c == 0), stop=(c == NC_ - 1))
                kvt = kv_p.tile([128, 512], bf16, tag=f"kv{sub}", name=f"kv{sub}")
                kv4 = kvt[:].rearrange("p (hp x s) -> p hp x s", hp=2, x=4, s=HS)
                psk4 = pskv2[:, 0:256].rearrange("p (hp e s) -> p hp e s",
                                                 hp=2, e=2, s=HS)
                psv4 = pskv2[:, 256:512].rearrange("p (hp e s) -> p hp e s",
                                                   hp=2, e=2, s=HS)
                # K lands at x={0,3}, V at x={1,2}: one strided copy each
                nc.scalar.copy(kv4[:, :, 0:4:3, :], psk4[:, :, :, :])
                nc.vector.tensor_copy(kv4[:, :, 1:3, :], psv4[:, :, :, :])
                kvsb.append(kvt)

            return kvsb

        def kpvp_pair(w, tb_odd, kvsb0, kvsb1):
            """kpT/vpT for a 2-t-block window in one PSUM group per head --
            halves the DVE drain adds. In phase A (wave 0) the idle pswt bank
            double-buffers the accumulators."""
            for hl in range(HW_):
                h = 4 * w + hl
                pp = pswt_p if (w == 0 and hl % 2 == 1) else pse_p
                pse = pp.tile([128, K], f32,
                              tag="pswt" if pp is pswt_p else "pse", name="pse")
                for s8 in range(8):
                    kvt = (kvsb0 if s8 < 4 else kvsb1)[s8 % 4]
                    nc.tensor.matmul(pse[:], kvt[:, hl * 128:(hl + 1) * 128],
                                     ett[hl][:, s8, :],
                                     start=(s8 == 0), stop=(s8 == 7))
                if tb_odd == 1:
                    nc.vector.tensor_copy(kvacc[h][:].bitcast(f32r), pse[:])
                else:
                    nc.vector.tensor_tensor(kvacc[h][:].bitcast(f32r), kvacc[h][:],
                                            pse[:], op=ALU.add)

        def phase15(w):
            """kpb (padded bf16) + vppair via PE transpose, wave w."""
            for pl in range(2):
                pr = 2 * w + pl
                h0, h1 = 2 * pr, 2 * pr + 1
                nc.gpsimd.memset(kpb[pr][HS:128, 0:K], 0.0)
                nc.gpsimd.memset(kpb[pr][0:HS, K:2 * K], 0.0)
                nc.scalar.copy(kpb[pr][0:HS, 0:K], kvacc[h0][0:HS, :])
                nc.scalar.copy(kpb[pr][HS:128, K:2 * K], kvacc[h1][HS:128, :])
                for h01 in range(2):
                    h = 2 * pr + h01
                    lo = (h01 == 0)
                    for j in range(2):
                        pvp = pse_p if j == 0 else pswt_p
                        psvp = pvp.tile([128, K], f32,
                                        tag="pse" if pvp is pse_p else "pswt",
                                        name="psvp")
                        nc.tensor.transpose(psvp[:, 0:128].bitcast(f32r),
                                            kvacc[h][:, j * 128:(j + 1) * 128].bitcast(f32r),
                                            ident[:])
                        vcols = psvp[:, 64:128] if lo else psvp[:, 0:64]
                        if h01 == 0:
                            nc.scalar.copy(vppair[pr][j][:, 0:HS], vcols)
                        else:
                            nc.vector.tensor_copy(vppair[pr][j][:, HS:128], vcols)

        def phase2_tb(w, tb, attTs, borrow=False):
            """attention (S/softmax/A) for wave w, t-block tb.
            attTs: list of 2 destination [128, TB] bf16 tiles (one per pair).
            borrow: double-buffer pss/pswt out of the idle phase-1 pools."""
            for pl in range(2):
                pr = 2 * w + pl
                att_dst = attTs[pl]
                # wt4: [128, sub(4), h01*2+j(4), 128] bf16, filled per sub
                wt4 = wt_p.tile([128, 4, 4, 128], bf16, tag="wt4", name="wt4")
                for sub in range(4):
                    tt = tb * 4 + sub
                    pssp = pskv2_p if (borrow and sub % 2 == 1) else pss_p
                    pss = pssp.tile([128, 2 * K], f32,
                                    tag="pskv2" if pssp is pskv2_p else "pss",
                                    name="pss")
                    nc.tensor.matmul(pss[:],
                                     qres[pr][:, tt * 128:(tt + 1) * 128],
                                     kpb[pr][:], start=True, stop=True)
                    if tt < 2:
                        for h01 in range(2):
                            nc.vector.tensor_tensor(
                                pss[:, h01 * K:(h01 + 1) * K],
                                pss[:, h01 * K:(h01 + 1) * K],
                                masksb[tt][:], op=ALU.add)
                    expw = ew_p.tile([128, 2 * K], bf16, tag="expw", name="expw")
                    z = zz_p.tile([128, 2], f32, tag="z", name="z")
                    rec = zz_p.tile([128, 2], f32, tag="rec", name="rec")
                    if sub % 2 == 0:
                        for h01 in range(2):
                            nc.scalar.activation(
                                expw[:, h01 * K:(h01 + 1) * K],
                                pss[:, h01 * K:(h01 + 1) * K],
                                AF.Exp, scale=float(SCALE),
                                accum_out=z[:, h01:h01 + 1])
                    else:
                        nc.scalar.activation(expw[:], pss[:], AF.Exp,
                                             scale=float(SCALE))
                        for h01 in range(2):
                            nc.vector.tensor_reduce(
                                z[:, h01:h01 + 1],
                                expw[:, h01 * K:(h01 + 1) * K],
                                axis=mybir.AxisListType.X, op=ALU.add)
                    nc.vector.reciprocal(rec[:], z[:])
                    pwp = pse_p if (borrow and sub % 2 == 0) else pswt_p
                    pswt = pwp.tile([128, 256], f32,
                                    tag="pse" if pwp is pse_p else "pswt",
                                    name="pswt")
                    psb = pswt[:].bitcast(bf16)
                    for h01 in range(2):
                        wn = wn_p.tile([128, K], bf16, tag="wn", name="wn")
                        nc.vector.tensor_scalar_mul(
                            wn[:], expw[:, h01 * K:(h01 + 1) * K],
                            rec[:, h01:h01 + 1])
                        for j in range(2):
                            nc.tensor.transpose(
                                psb[:, (h01 * 2 + j) * 128:(h01 * 2 + j + 1) * 128],
                                wn[:, j * 128:(j + 1) * 128],
                                identb[:])
                    nc.vector.tensor_copy(wt4[:, sub, :, :], psb.rearrange(
                        "p (hj i) -> p hj i", hj=4))
                for h01 in range(2):
                    pso = pso_p.tile([HS, TB], f32, tag="pso", name="pso")
                    for j in range(2):
                        nc.tensor.matmul(
                            pso[:],
                            vppair[pr][j][:, h01 * HS:(h01 + 1) * HS],
                            wt4[:, :, h01 * 2 + j, :],
                            start=(j == 0), stop=(j == 1))
                    if h01 == 0:
                        nc.vector.tensor_copy(att_dst[0:HS, :], pso[:])
                    else:
                        atmp = atmp_p.tile([HS, TB], bf16, tag="atmp", name="atmp")
                        nc.scalar.copy(atmp[:], pso[:])
                        nc.sync.dma_start(att_dst[HS:128, :], atmp[:])

        def proj_tb(tb, attT4):
            t0 = tb * TB
            last = (tb == NTB - 1)
            for sub in range(4):
                outsb = out_p.tile([128, C], bf16, tag="outsb", name="outsb")
                row = t0 + sub * 128
                for n in range(2):
                    psp = psq_p.tile([128, 512], f32, tag="psq", name="psp")
                    for ci in range(4):
                        nc.tensor.matmul(psp[:],
                                         attT4[ci][:, sub * 128:(sub + 1) * 128],
                                         wpt[:, ci * C + n * 512:ci * C + (n + 1) * 512],
                                         start=(ci == 0), stop=(ci == 3))
                    if n == 0:
                        nc.scalar.copy(outsb[:, 0:512], psp[:])
                    else:
                        nc.vector.tensor_copy(outsb[:, 512:1024], psp[:])
                    if last:
                        # drain the final rows in halves on alternating
                        # queues so the tail DMA starts as soon as each
                        # half is staged
                        eng = nc.sync if n == 0 else nc.scalar
                        eng.dma_start(O[row:row + 128, n * 512:(n + 1) * 512],
                                      outsb[:, n * 512:(n + 1) * 512])
                if not last:
                    nc.sync.dma_start(O[row:row + 128, :], outsb[:])

        for _rep in range(max(1, repeat)):
            # halves so the first Q matmuls start ~1us earlier; wpt is
            # first read in phase C so it loads during phase B instead of
            # stealing phase-A startup DMA bandwidth
            wq8f = wq8[:].rearrange("p cp two m -> p (cp two m)")
            nc.scalar.dma_start(wq8f[:, 0:2048], WQ8[:, 0:2048])
            nc.scalar.dma_start(wq8f[:, 2048:], WQ8[:, 2048:])
            nc.scalar.dma_start(wk8[:].rearrange("p cp two m -> p (cp two m)"),
                                WK8[:])
            nc.scalar.dma_start(wvt[:], WV[:])
            if phases & 1:
                # phase A: wave-0 projections
                kvprev = None
                for tb in range(NTB):
                    kvsb = phase1_tb(0, tb)
                    if tb % 2 == 1:
                        kpvp_pair(0, tb, kvprev, kvsb)
                    kvprev = kvsb
                phase15(0)
            if not (phases & 1):
                nc.scalar.dma_start(wpt[:], WPT[:])
            if phases == 3:
                # phase B: wave-1 projections interleaved with wave-0 attention
                nc.scalar.dma_start(wpt[:], WPT[:])
                kvprev = None
                for tb in range(NTB):
                    phase2_tb(0, tb, attw0[tb])
                    kvsb = phase1_tb(1, tb)
                    if tb % 2 == 1:
                        kpvp_pair(1, tb, kvprev, kvsb)
                    kvprev = kvsb
                phase15(1)
            if phases & 2:
                # phase C: wave-1 attention + full projection (proj trails by
                # one t-block so it fills the next block's softmax stalls)
                prev = None
                for tb in range(NTB):
                    attT1 = [at_p.tile([128, TB], bf16, tag=f"attT{p}",
                                       name=f"attT{p}") for p in range(2)]
                    phase2_tb(1, tb, attT1, borrow=(phases == 3))
                    if prev is not None:
                        proj_tb(tb - 1, [attw0[tb - 1][0], attw0[tb - 1][1],
                                         prev[0], prev[1]])
                    prev = attT1
                proj_tb(NTB - 1, [attw0[NTB - 1][0], attw0[NTB - 1][1],
                                  prev[0], prev[1]])

    nc.finalize()
    return nc


_NC_CACHE = {}


def _get_program(phases=3):
    if phases not in _NC_CACHE:
        _NC_CACHE[phases] = _build_program(phases)
    return _NC_CACHE[phases]


def _pack_w(w_core):
    """[C, 512] -> [128, 8*512] with chunk c at cols c*512."""
    return np.ascontiguousarray(
        w_core.reshape(NC_, 128, 512).transpose(1, 0, 2).reshape(128, NC_ * 512))


def _pack_w8(w_core):
    """[C, 512] fp8 -> [128, cp(4), two(2), 512] flat [128, 4096]."""
    return np.ascontiguousarray(
        w_core.reshape(4, 2, 128, 512).transpose(2, 0, 1, 3).reshape(128, 4096))


def _make_in_maps(x, WQ, WK, WV, E, Wp):
    import ml_dtypes
    f8np = ml_dtypes.float8_e4m3
    xr = np.transpose(np.asarray(x), (0, 2, 1)).astype(ml_dtypes.bfloat16)  # [B, C, T]
    xr8 = np.transpose(np.asarray(x), (0, 2, 1)).astype(f8np)
    wq_full = np.transpose(np.asarray(WQ), (1, 0, 2)).astype(f8np)
    wk_full = np.transpose(np.asarray(WK), (1, 0, 2)).astype(f8np)
    wv_full = np.transpose(np.asarray(WV), (1, 0, 2)).astype(ml_dtypes.bfloat16)
    er = np.asarray(E).astype(ml_dtypes.bfloat16)                 # [H, B, T, K]
    wpt_full = np.ascontiguousarray(np.asarray(Wp).T).astype(ml_dtypes.bfloat16)

    msk = np.zeros((2, 128, K), np.float32)
    for i in range(2):
        t_idx = i * 128 + np.arange(128)[:, None]
        msk[i] = np.where(np.arange(K)[None, :] <= t_idx, 0.0, -1e30)
    idn = np.eye(128, dtype=np.float32)

    in_maps = []
    for core in range(8):
        b, g = core // 2, core % 2
        hs = slice(g * HL, (g + 1) * HL)
        wpt_core = wpt_full[g * 512:(g + 1) * 512, :]              # [512, 1024]
        wpt_packed = np.ascontiguousarray(
            wpt_core.reshape(4, 128, C).transpose(1, 0, 2).reshape(128, 4 * C))
        xtw = np.ascontiguousarray(
            xr[b].reshape(NC_, 128, 4, 1024).transpose(2, 1, 0, 3)
            .reshape(4, 128, NC_ * 1024))
        x8w = np.ascontiguousarray(
            xr8[b].reshape(4, 2, 128, 4, 1024).transpose(3, 2, 0, 1, 4)
            .reshape(4, 128, 8 * 1024))
        edw = np.ascontiguousarray(
            er[hs, b].reshape(HL, 4, 8, 128, K).transpose(0, 1, 3, 2, 4)
            .reshape(HL, 4, 128, 8 * K))
        in_maps.append({
            "XTW": xtw,
            "X8W": x8w,
            "WQ8": _pack_w8(np.ascontiguousarray(wq_full[:, hs, :]).reshape(C, HL * HS)),
            "WK8": _pack_w8(np.ascontiguousarray(wk_full[:, hs, :]).reshape(C, HL * HS)),
            "WV": _pack_w(np.ascontiguousarray(wv_full[:, hs, :]).reshape(C, HL * HS)),
            "EDW": edw,
            "WPT": wpt_packed,
            "MSK": msk,
            "IDN": idn,
        })
    return in_maps


def _run(x, WQ, WK, WV, E, Wp, bp, trace=False):
    nc = _get_program()
    in_maps = _make_in_maps(x, WQ, WK, WV, E, Wp)
    kw = {}
    if trace:
        kw = dict(trace=True, trace_cores=[0])
    res = run_bass_kernel_spmd(nc, in_maps, list(range(8)), **kw)
    out = np.zeros((B, T, C), np.float32)
    for b in range(B):
        out[b] = (res.results[2 * b]["O"].astype(np.float32)
                  + res.results[2 * b + 1]["O"].astype(np.float32))
    out += np.asarray(bp, np.float32)[None, None, :]
    return out, res


def kernel(x, WQ, WK, WV, E, Wp, bp):
    out, _ = _run(x, WQ, WK, WV, E, Wp, bp, trace=False)
    return out


def kernel_traced(x, WQ, WK, WV, E, Wp, bp):
    out, res = _run(x, WQ, WK, WV, E, Wp, bp, trace=True)
    return out, res

